# revision 3
# baseline (speedup 1.0000x reference)
"""Trainium2 Bass kernel for the ChaosModulator recurrence.

Math (per (b,c) sequence, t = 0..4095):
    sigma_t = 3.5*z*(1-z) + 0.5*x_t
    z'      = 0.5*z + 0.25*(1 + tanh(sigma_t))        (clip is a no-op: z' in (0,1))
    u_t     = 0.5*x_t + 0.5*(2*z' - 1)

Reformulated with w = 2z-1 and state s_t = w_t + h_t (so w_{t+1} = 0.5*s_t):
    e_t = 0.5*x_t - (7/32)*s_{t-1}^2
    h_t = tanh(e_t + 0.875)
    s_t = 0.5*s_{t-1} + h_t
    u_t = 0.25*s_t + 0.5*x_t

The map contracts with factor ~0.5/step, so each 128-step time block can be
computed independently after a 32-step warmup from an arbitrary state
(validated: fp64-exact at W=32).  This turns the serial t-loop into
32 independent chains per sequence -> wide [128 x 128] per-step ops.

Sharding: batch dim b (32) split 4-per-core across 8 cores; per core
2048 sequences x 4096 steps.
"""

import numpy as np

import concourse.bacc as bacc
import concourse.mybir as mybir
from concourse.bass_utils import run_bass_kernel_spmd
from concourse.tile import TileContext

F32 = mybir.dt.float32
P = 128             # SBUF partitions
G = 16              # sequence groups per core (2048 = G*P)
T = 4096
B = 128             # output steps per block
W = 32              # warmup steps per block
L = B + W           # chain length
NBLK = T // B       # 32 blocks
BLK_PER_BATCH = 8
NBATCH = NBLK // BLK_PER_BATCH   # 4
NCOLS = BLK_PER_BATCH * G        # 128 chain-columns per batch
NSEQ = P * G        # 2048 sequences per core
NCORES = 8

_MULT = mybir.AluOpType.mult
_ADD = mybir.AluOpType.add


def _build_nc():
    nc = bacc.Bacc("TRN2", target_bir_lowering=False, debug=False)

    x = nc.dram_tensor("x", [NSEQ, T], F32, kind="ExternalInput")
    z0 = nc.dram_tensor("z0", [NSEQ], F32, kind="ExternalInput")
    u = nc.dram_tensor("u", [NSEQ, T], F32, kind="ExternalOutput")

    xr = x[:, :].rearrange("(g p) t -> p g t", p=P)    # [128, 16, 4096]
    ur = u[:, :].rearrange("(g p) t -> p g t", p=P)
    z0r = z0[:].rearrange("(g p) -> p g", p=P)         # [128, 16]

    with TileContext(nc) as tc:
        with (
            tc.tile_pool(name="xp", bufs=2) as xp,
            tc.tile_pool(name="sp", bufs=2) as sp,
            tc.tile_pool(name="cp", bufs=1) as cp,
        ):
            z0_t = cp.tile([P, G], F32)
            nc.sync.dma_start(out=z0_t[:, :], in_=z0r)
            # s_init = 4*z0 - 2  (so that w_0 = 0.5*s_init = 2*z0 - 1)
            s_init = cp.tile([P, G], F32)
            nc.vector.tensor_scalar(
                out=s_init[:, :], in0=z0_t[:, :],
                scalar1=4.0, scalar2=-2.0, op0=_MULT, op1=_ADD,
            )
            # per-partition bias for tanh(e + 0.875)
            bias_t = cp.tile([P, 1], F32)
            nc.vector.memset(bias_t[:, :], 0.875)

            for bt in range(NBATCH):
                # X tile: [c][k] layout, c = nl*G + g, k = chain step.
                Xt = xp.tile([P, NCOLS * L], F32, name=f"X{bt}", tag="X")
                Xv = Xt.rearrange("p (c k) -> p c k", k=L)

                for nl in range(BLK_PER_BATCH):
                    n = bt * BLK_PER_BATCH + nl
                    cs = nl * G
                    if n == 0:
                        # block 0 starts at t=-W: pad warmup with zeros
                        nc.vector.memset(Xv[:, cs:cs + G, 0:W], 0.0)
                        nc.sync.dma_start(
                            out=Xv[:, cs:cs + G, W:L], in_=xr[:, :, 0:B]
                        )
                    else:
                        t0 = n * B - W
                        nc.sync.dma_start(
                            out=Xv[:, cs:cs + G, :], in_=xr[:, :, t0:t0 + L]
                        )

                # xh = 0.5*x in place (fp32 tensor_scalar runs in 2x mode)
                nc.vector.tensor_scalar_mul(Xt[:, :], Xt[:, :], 0.5)

                h_t = sp.tile([P, NCOLS], F32, name=f"h{bt}", tag="h")
                p_t = sp.tile([P, NCOLS], F32, name=f"p{bt}", tag="pp")
                e_t = [
                    sp.tile([P, NCOLS], F32, name=f"e{bt}_{i}", tag=f"e{i}")
                    for i in range(2)
                ]
                s_t = [
                    sp.tile([P, NCOLS], F32, name=f"s{bt}_{i}", tag=f"s{i}")
                    for i in range(2)
                ]

                nc.vector.memset(s_t[0][:, :], 0.0)
                # e_0 = xh_0 - (7/32)*0^2 = xh_0
                nc.vector.tensor_copy(out=e_t[0][:, :], in_=Xv[:, :, 0])

                for k in range(L):
                    cur, nxt = k % 2, (k + 1) % 2
                    # h = tanh(e + 0.875)
                    nc.scalar.activation(
                        out=h_t[:, :], in_=e_t[cur][:, :],
                        func=mybir.ActivationFunctionType.Tanh,
                        bias=bias_t[:, :], scale=1.0,
                    )
                    # s' = 0.5*s + h
                    nc.vector.scalar_tensor_tensor(
                        out=s_t[nxt][:, :], in0=s_t[cur][:, :], scalar=0.5,
                        in1=h_t[:, :], op0=_MULT, op1=_ADD,
                    )
                    if bt == 0 and k == W - 1:
                        # block 0: replace warmup state with the true z0 state
                        nc.vector.tensor_copy(
                            out=s_t[nxt][:, 0:G], in_=s_init[:, :]
                        )
                    if k < L - 1:
                        # e' = xh_{k+1} - (7/32)*s'^2
                        nc.vector.tensor_mul(
                            out=p_t[:, :], in0=s_t[nxt][:, :], in1=s_t[nxt][:, :]
                        )
                        nc.vector.scalar_tensor_tensor(
                            out=e_t[nxt][:, :], in0=p_t[:, :], scalar=-0.21875,
                            in1=Xv[:, :, k + 1], op0=_MULT, op1=_ADD,
                        )
                    if k >= W:
                        # u = 0.25*s' + xh_k  -> store into dead slot k-W
                        nc.vector.scalar_tensor_tensor(
                            out=Xv[:, :, k - W], in0=s_t[nxt][:, :], scalar=0.25,
                            in1=Xv[:, :, k], op0=_MULT, op1=_ADD,
                        )

                for nl in range(BLK_PER_BATCH):
                    n = bt * BLK_PER_BATCH + nl
                    cs = nl * G
                    nc.sync.dma_start(
                        out=ur[:, :, n * B:(n + 1) * B],
                        in_=Xv[:, cs:cs + G, 0:B],
                    )

    nc.compile()
    return nc


_NC = None


def _get_nc():
    global _NC
    if _NC is None:
        _NC = _build_nc()
    return _NC


def kernel(x: np.ndarray, z0: np.ndarray) -> np.ndarray:
    x = np.ascontiguousarray(x, dtype=np.float32)      # (32, 512, 4096)
    z0 = np.ascontiguousarray(z0, dtype=np.float32)    # (32, 512)
    nc = _get_nc()

    in_maps = []
    for i in range(NCORES):
        xs = np.ascontiguousarray(x[4 * i:4 * (i + 1)].reshape(NSEQ, T))
        zs = np.ascontiguousarray(z0[4 * i:4 * (i + 1)].reshape(NSEQ))
        in_maps.append({"x": xs, "z0": zs})

    res = run_bass_kernel_spmd(nc, in_maps, core_ids=list(range(NCORES)))
    out = np.empty((32, 512, T), np.float32)
    for i in range(NCORES):
        out[4 * i:4 * (i + 1)] = res.results[i]["u"].reshape(4, 512, T)
    return out
